# revision 21
# baseline (speedup 1.0000x reference)
"""Trainium2 kernel for nn_Graph_41609643163904.

The reference op is a sequential per-cell scatter sweep over a 48x48 grid
(x outer, y inner): read center v, zero it, add v*W[y,x] to the 5x5
neighborhood.  Every step is linear in the grid, so the sweep is a fixed
linear operator M (2304x2304) of the weights.

This version exploits the sweep's sequentially-semiseparable structure:
all influence crossing grid-column X flows through the 88-dim interface
s(X) = [v(X); v(X-1)] (v = per-cell fired values, 44 per column).  In
x-major layout, each 128-row output tile j decomposes EXACTLY as

    out_j = sum_k  nearblk[j,k] @ g0_ktile[k]   (cols > cut_j)
          + U_j @ s(cut_j)                      (everything to the left)

The cut set is globally optimized: instead of one cut per output tile
(16 cuts), a brute-forced 10-cut set shares states between neighboring
tiles, trading a few extra near k-tiles for 6 fewer chain steps.  That
cuts the matmul count 206 -> ~176 (x2 512-wide halves) and the chain g0
DMA stack from 4.2MB to 2.6MB.  All operands bf16 (tolerance 2e-2;
end-to-end lands ~5e-3), halving DMA.  Data-parallel over the 8192-
sample batch across 8 cores, no comm.

Hardware lessons baked into the schedule (each worth ~5-20us):
  * every dma_start costs ~0.7us of its sequencer: loads are merged into
    a few big DMAs; x is pre-tiled on host to [128, 18*BS] so each
    multi-k-tile chunk is ONE dma with contiguous per-partition runs
  * SDMA engines round-robin the *queues* at packet granularity, so
    bytes must be balanced per queue: all loads ride SP (x chunks
    interleaved with chain stacks in first-use order), operator chunks
    ride ACT, stores ride SWDGE -- moving the chain stacks to SWDGE
    starves the x stream (measured +21us)
  * all states live in ONE [104, 10*BS] tile so the per-step overflow
    loads merge into a single DMA (rows 88:104, slot i-1 feeds step i)
  * PSUM->SBUF copies cost ~1.2us nearly flat in size: one big copy per
    PSUM tile, out-copies on DVE, state copies on ACT (disjoint queues)
  * chain step i is emitted right before its first consumer out_group,
    so chain-data DMA latency has maximal slack
  * ~40 dummy matmuls at t=0 keep the PE busy through the HAM 3.4us
    activity window (memset on DVE, not ACT, so warm-up does not wait
    for the activation-table load); first real matmuls then run at
    2.4GHz instead of 1.2
  * final stores ride the by-then-idle SP/ACT HWDGE rings instead of
    queueing behind the SWDGE emission path
"""

import os

import numpy as np

SIZE = 48
D = 2
KS = 5
N = SIZE * SIZE          # 2304
B = 8192
NCORES = 8
BS = B // NCORES         # 1024 samples per core
P = 128
NT = N // P              # 18 tiles of 128
MW = 512                 # matmul moving-dim (PSUM bank)
NM = BS // MW            # 2 halves

# brute-forced cut set (see docstring): 10 cuts shared across out tiles
_XS = [4, 7, 12, 15, 20, 23, 28, 31, 36, 39]
_NSTEP = len(_XS)
OVS = 16                 # small-step overflow rows (ride in the sb piece)


# ---------------------------------------------------------------- plan ----

def _plan():
    js = []
    for j in range(NT):
        r0 = P * j
        jc_lo, jc_hi = r0 // SIZE, (r0 + P - 1) // SIZE
        if j < 2:
            cut, step = None, None
            ncol_lo = 0
        else:
            cut = max(x for x in _XS if x <= jc_lo - 1)
            step = _XS.index(cut)
            ncol_lo = cut + 1
        ncol_hi = min(jc_hi + 2, SIZE - 1)
        kt_lo = (SIZE * ncol_lo) // P
        kt_hi = (SIZE * (ncol_hi + 1) - 1) // P
        js.append(dict(j=j, jc_lo=jc_lo, jc_hi=jc_hi, ncol_lo=ncol_lo,
                       kts=list(range(kt_lo, kt_hi + 1)),
                       cut=cut, step=step))
    steps = []
    nbig = 0
    for i, X in enumerate(_XS):
        c0 = 2 if i == 0 else _XS[i - 1] + 1
        ng = SIZE * (X - c0 + 1)
        ka = min(ng, P)
        ov = max(ng - P, 0)
        big = ov > (0 if i == 0 else 40 - 0)  # i==0 has no sb; use gb there
        # i == 0: overflow always goes to its own gb piece (no sb piece)
        use_gb = (i == 0 and ov > 0) or ov > 40
        bslot = None
        if use_gb and i > 0:
            bslot = nbig
            nbig += 1
        steps.append(dict(i=i, X=X, c0=c0, ng=ng, ka=ka, ov=ov,
                          use_gb=use_gb, bslot=bslot))
    return js, steps, nbig


_JS, _STEPS, _NBIG = _plan()


def _step_pieces(i):
    """chain step i matmul pieces in accumulation order (state last).
    kinds: ga (g0 rows 0:ka), gb (overflow via gb0/gbig), sb (state,
    plus small overflow rows riding in stbig[88:88+ov])."""
    s = _STEPS[i]
    out = [("ga", s["ka"])]
    if s["use_gb"]:
        out.append(("gb", s["ov"]))
    if i > 0:
        k = 88 + (s["ov"] if (s["ov"] and not s["use_gb"]) else 0)
        out.append(("sb", k))
    return out


# ------------------------------------------------------- host operators ----

def _build_M_V(weights):
    """Composed operator M (N,N) and firing-value gradient rows V (1936,N),
    fp64, in the original y-major flattening."""
    M = np.eye(N, dtype=np.float64)
    V = np.zeros((44 * 44, N), dtype=np.float64)
    w = weights.astype(np.float64)
    for x in range(D, SIZE - D):
        for y in range(D, SIZE - D):
            c = y * SIZE + x
            wc = w[y, x]
            rc = M[c].copy()
            V[(x - D) * 44 + (y - D)] = rc
            for dy in range(-D, D + 1):
                r0 = c + dy * SIZE - D
                wrow = wc[dy + D]
                if dy == 0:
                    M[r0:r0 + D] += np.outer(wrow[:D], rc)
                    M[r0 + D + 1:r0 + KS] += np.outer(wrow[D + 1:], rc)
                else:
                    M[r0:r0 + KS] += np.outer(wrow, rc)
            M[c] = wc[D, D] * rc
    return M, V


def _xmajor_idx():
    n = np.arange(N)
    return (n % SIZE) * SIZE + n // SIZE


def _srows(X):
    return np.concatenate([(X - 2) * 44 + np.arange(44),
                           (X - 3) * 44 + np.arange(44)])


def _build_operators(weights):
    M, V = _build_M_V(weights)
    idx = _xmajor_idx()
    Mx = M[np.ix_(idx, idx)]
    Vx = V[:, idx]
    ops = {}
    for d in _JS:
        j = d["j"]
        jr = slice(P * j, P * j + P)
        e_lo = SIZE * d["ncol_lo"]
        for kt in d["kts"]:
            blk = Mx[jr, P * kt:P * kt + P].copy()
            cols = np.arange(P * kt, P * kt + P)
            blk[:, cols < e_lo] = 0.0
            ops[("near", j, kt)] = blk.T        # lhsT (K=128, M=128)
        if d["cut"] is not None:
            X = d["cut"]
            sf = Vx[_srows(X), :SIZE * (X + 1)]
            F = Mx[jr, :SIZE * (X + 1)]
            U, _, _, _ = np.linalg.lstsq(sf.T, F.T, rcond=None)
            ops[("far", j)] = U                 # lhsT (K=88, M=128)
    for s in _STEPS:
        i, X, c0 = s["i"], s["X"], s["c0"]
        Binj = Vx[_srows(X), SIZE * c0:SIZE * (X + 1)]  # full columns
        ka, ov = s["ka"], s["ov"]
        ops[("chain_ga", i)] = Binj.T[:ka]
        if s["use_gb"]:
            ops[("chain_gb", i)] = Binj.T[ka:ka + ov]
        if i > 0:
            Xp = _XS[i - 1]
            sf_p = Vx[_srows(Xp), :SIZE * (Xp + 1)]
            tgt = Vx[_srows(X), :SIZE * (Xp + 1)]
            T, _, _, _ = np.linalg.lstsq(sf_p.T, tgt.T, rcond=None)
            if ov and not s["use_gb"]:
                ops[("chain_sb", i)] = np.vstack([T, Binj.T[ka:ka + ov]])
            else:
                ops[("chain_sb", i)] = T
    return ops


# ------------------------------------------------ emission/consumption ----

def _emission():
    """(kind, idx) sequence: out groups with chain steps spliced right
    before their first consumer."""
    seq = [("og", 0), ("og", 1)]
    done = set()
    for j in range(2, NT):
        st = _JS[j]["step"]
        if st is not None and st not in done:
            # emit any not-yet-emitted steps up to st (chain is sequential)
            for i in range(len(done), st + 1):
                seq.append(("ch", i))
                done.add(i)
        seq.append(("og", j))
    return seq


_EMIT = _emission()


def _pack_layout():
    """Column ranges in the packed wt tensor, in PE consumption order."""
    off = 0
    lay = {}

    def put(key, cols):
        nonlocal off
        lay[key] = (off, cols)
        off += cols

    for kind, idx in _EMIT:
        if kind == "og":
            d = _JS[idx]
            for kt in d["kts"]:
                put(("near", idx, kt), P)
            if d["cut"] is not None:
                put(("far", idx), P)
        else:
            for pc, _k in _step_pieces(idx):
                put((f"chain_{pc}", idx), 88)
    return lay, off


_LAY, _TOTC = _pack_layout()


def _pack_ops(ops):
    wt = np.zeros((P, _TOTC), dtype=np.float32)
    for d in _JS:
        j = d["j"]
        for kt in d["kts"]:
            o, c = _LAY[("near", j, kt)]
            wt[:, o:o + c] = ops[("near", j, kt)]
        if d["cut"] is not None:
            o, c = _LAY[("far", j)]
            wt[:88, o:o + P] = ops[("far", j)]
    for s in _STEPS:
        i = s["i"]
        for pc, k in _step_pieces(i):
            o, _ = _LAY[(f"chain_{pc}", i)]
            blk = ops[(f"chain_{pc}", i)]
            assert blk.shape[0] == k, (i, pc, blk.shape, k)
            wt[:k, o:o + 88] = blk
    return wt


# fetch groups in consumption order: one group per _EMIT entry
def _fetch_groups():
    gs = []
    for kind, idx in _EMIT:
        if kind == "og":
            d = _JS[idx]
            keys = [("near", idx, kt) for kt in d["kts"]]
            if d["cut"] is not None:
                keys.append(("far", idx))
            gs.append((f"og{idx}", keys))
        else:
            gs.append((f"ch{idx}",
                       [(f"chain_{pc}", idx)
                        for pc, _k in _step_pieces(idx)]))
    return gs


_FETCH = _fetch_groups()


# ------------------------------------------------------------- device ----

def _build_device_kernel():
    import concourse.mybir as mybir
    from concourse import bacc
    from concourse.tile import TileContext

    f32 = mybir.dt.float32
    bf16 = mybir.dt.bfloat16

    nc = bacc.Bacc()
    # xTc: host pre-tiled [128, 18*BS] -- k-tile k at free block k, so a
    # multi-k-tile chunk load is one plain 2D slice with contiguous
    # per-partition runs.
    xTc = nc.dram_tensor("xTc", [P, NT * BS], bf16, kind="ExternalInput")
    wt = nc.dram_tensor("wt", [P, _TOTC], bf16, kind="ExternalInput")
    # host-packed chain g0 stacks: step i at free block i
    xga = nc.dram_tensor("xga", [P, _NSTEP * BS], bf16, kind="ExternalInput")
    # small overflow rows (step i block i; zeros for big steps)
    xgb = nc.dram_tensor("xgb", [OVS, _NSTEP * BS], bf16,
                         kind="ExternalInput")
    # big-step overflow rows (112 each), bslot order
    xgg = nc.dram_tensor("xgg", [P - OVS, max(_NBIG, 1) * BS], bf16,
                         kind="ExternalInput")
    outT = nc.dram_tensor("outT", [N, BS], bf16, kind="ExternalOutput")

    XCH = [2, 2, 3, 3, 4, 4]     # x k-tiles per SBUF chunk (one DMA each)
    GACH = [4, 3, 3]             # chain-ga steps per SBUF chunk
    WSIZES = [3, 5, 5, 5, 5, 5]  # w chunk sizes in fetch groups
    NWARM = 70

    with TileContext(nc) as tc:
        with (
            tc.tile_pool(name="xpool", bufs=1) as xpool,
            tc.tile_pool(name="spool", bufs=1) as spool,
            tc.tile_pool(name="wpool", bufs=1) as wpool,
            tc.tile_pool(name="opool", bufs=6) as opool,
            tc.tile_pool(name="pso", bufs=4, space="PSUM") as pso,
            tc.tile_pool(name="pss", bufs=2, space="PSUM") as pss,
        ):
            # Engine / ring roles (SDMA round-robins the rings per packet):
            #   sync  (SP HWDGE):  x chunks + chain stacks, last stores
            #   scalar (ACT HWDGE): operator chunks; state copies
            #   gpsimd (SWDGE):    output stores
            #   vector (DVE):      out PSUM->SBUF copies
            kt0_of = []
            acc = 0
            for nk in XCH:
                kt0_of.append(acc)
                acc += nk
            assert acc == NT

            def chunk_of(kt):
                for ci in range(len(XCH) - 1, -1, -1):
                    if kt >= kt0_of[ci]:
                        return ci
                raise AssertionError

            xch = [xpool.tile([P, nk * BS], bf16, tag=f"x{ci}", name=f"x{ci}")
                   for ci, nk in enumerate(XCH)]

            def issue_xc(ci):
                k0, nk = kt0_of[ci], XCH[ci]
                nc.sync.dma_start(out=xch[ci][:, :],
                                  in_=xTc[:, k0 * BS:(k0 + nk) * BS])

            def x_ap(kt, c0, c1):
                ci = chunk_of(kt)
                off = (kt - kt0_of[ci]) * BS
                return xch[ci][:, off + c0:off + c1]

            ga0_of = []
            acc = 0
            for ns in GACH:
                ga0_of.append(acc)
                acc += ns
            assert acc == _NSTEP

            def gchunk_of(i):
                for ci in range(len(GACH) - 1, -1, -1):
                    if i >= ga0_of[ci]:
                        return ci
                raise AssertionError

            gach = [xpool.tile([P, ns * BS], bf16, tag=f"ga{q}", name=f"ga{q}")
                    for q, ns in enumerate(GACH)]

            def issue_ga(ci):
                s0, ns = ga0_of[ci], GACH[ci]
                nc.sync.dma_start(out=gach[ci][:, :],
                                  in_=xga[:, s0 * BS:(s0 + ns) * BS])

            # step-0 overflow rows (K<=16)
            gb0 = xpool.tile([OVS, BS], bf16, tag="gb0", name="gb0")
            # big-step overflow rows
            gbig = xpool.tile([P - OVS, max(_NBIG, 1) * BS], bf16,
                              tag="gbig", name="gbig")
            # all states in one tile: rows 0:88 = state i at slot i, rows
            # 88:88+OVS = step i+1's small-overflow rows (one merged DMA)
            stbig = spool.tile([88 + OVS, _NSTEP * BS], bf16,
                               tag="st", name="st")

            # operator stream: merged chunk DMAs, consumption order
            wslot = {}
            wchunks = []
            nchunks = len(WSIZES)
            woff = [0]
            for s in WSIZES:
                woff.append(woff[-1] + s)
            assert woff[-1] == len(_FETCH), (woff[-1], len(_FETCH))
            for ci in range(nchunks):
                grp = _FETCH[woff[ci]:woff[ci + 1]]
                keys = [k for _, ks in grp for k in ks]
                o0 = _LAY[keys[0]][0]
                cols = sum(_LAY[k][1] for k in keys)
                wtile = wpool.tile([P, 3584], bf16, tag=f"w{ci}",
                                   name=f"w{ci}")
                wchunks.append((wtile, o0, cols))
                for k in keys:
                    wslot[k] = (wtile, _LAY[k][0] - o0)

            def issue_w(ci, eng=None):
                wtile, o0, cols = wchunks[ci]
                eng = eng or nc.scalar
                eng.dma_start(out=wtile[:, :cols],
                              in_=wt[:, o0:o0 + cols])

            def w_ap(key, kk):
                wtile, o = wslot[key]
                m = 88 if key[0].startswith("chain") else P
                return wtile[0:kk, o:o + m]

            def out_group(j):
                d = _JS[j]
                items = [("near", kt) for kt in d["kts"]]
                if d["cut"] is not None:
                    items.append(("far", None))
                # one 1-bank PSUM tile per 512-half: half-m copies start
                # as soon as that half's accumulation finishes
                psm = [pso.tile([P, MW], f32, tag="o", name=f"ps{j}_{m}")
                       for m in range(NM)]
                s0 = (d["step"] or 0) * BS
                for it, (kind, kt) in enumerate(items):
                    first, last = it == 0, it == len(items) - 1
                    for m in range(NM):
                        if kind == "near":
                            lhsT = w_ap(("near", j, kt), P)
                            rhs = x_ap(kt, m * MW, (m + 1) * MW)
                        else:
                            lhsT = w_ap(("far", j), 88)
                            rhs = stbig[0:88, s0 + m * MW:s0 + (m + 1) * MW]
                        nc.tensor.matmul(psm[m][:, :],
                                         lhsT=lhsT, rhs=rhs,
                                         start=first, stop=last)
                oc = opool.tile([P, BS], bf16, tag="o", name=f"oc{j}")
                if j == NT - 1:  # final tile: parallel half copies + stores
                    nc.vector.tensor_copy(oc[:, 0:MW], psm[0][:, :])
                    nc.scalar.copy(oc[:, MW:BS], psm[1][:, :])
                    nc.sync.dma_start(out=outT[P * j:P * j + P, 0:MW],
                                      in_=oc[:, 0:MW])
                    nc.scalar.dma_start(out=outT[P * j:P * j + P, MW:BS],
                                        in_=oc[:, MW:BS])
                else:
                    nc.vector.tensor_copy(oc[:, 0:MW], psm[0][:, :])
                    nc.vector.tensor_copy(oc[:, MW:BS], psm[1][:, :])
                    if j >= NT - 3:
                        nc.sync.dma_start(out=outT[P * j:P * j + P, 0:MW],
                                          in_=oc[:, 0:MW])
                        nc.sync.dma_start(out=outT[P * j:P * j + P, MW:BS],
                                          in_=oc[:, MW:BS])
                    else:
                        nc.gpsimd.dma_start(out=outT[P * j:P * j + P, :],
                                            in_=oc[:])

            def chain_step(i):
                s = _STEPS[i]
                ps = pss.tile([88, BS], f32, tag="s", name=f"pss{i}")
                pieces = _step_pieces(i)
                for it, (pc, kdim) in enumerate(pieces):
                    first, last = it == 0, it == len(pieces) - 1
                    if pc == "ga":
                        ci = gchunk_of(i)
                        rt, base = gach[ci], (i - ga0_of[ci]) * BS
                    elif pc == "gb":
                        if i == 0:
                            rt, base = gb0, 0
                        else:
                            rt, base = gbig, s["bslot"] * BS
                    else:
                        rt, base = stbig, (i - 1) * BS
                    for m in range(NM):
                        nc.tensor.matmul(
                            ps[:, m * MW:(m + 1) * MW],
                            lhsT=w_ap((f"chain_{pc}", i), kdim),
                            rhs=rt[0:kdim, base + m * MW:base + (m + 1) * MW],
                            start=first, stop=last)
                nc.scalar.copy(stbig[0:88, i * BS:(i + 1) * BS], ps[:])

            # ---------------- emission ----------------
            # PE warm-up: un-throttle HAM during the initial DMA window.
            # memset on DVE so warm-up does not wait for the ACT
            # activation-table load.
            wu = spool.tile([P, P], bf16, tag="warm", name="warm")
            nc.vector.memset(wu[:], 0.0)
            pwu = pso.tile([P, MW], f32, tag="o", name="pswarm")
            for _ in range(NWARM):
                nc.tensor.matmul(pwu[:, 0:64], lhsT=wu[:], rhs=wu[:, 0:64],
                                 start=True, stop=True)

            # Loads.  The SP and ACT HWDGE rings generate descriptors
            # concurrently (~0.65us per dma_start each), but only 8 HWDGE
            # sem lanes exist globally: the (8+k)th HWDGE dma_start gates
            # on the k-th one's first consumer.  So the x / chain-ga
            # stream rides SP in consumption order, the operator chunks
            # ride ACT, and the chain-overflow stacks ride the SWDGE ring
            # (own generator + own sems, idle until the first store).
            issue_xc(0)          # kt 0-1   (og0)
            issue_w(0)
            issue_xc(1)          # kt 2-3   (og1-2)
            issue_ga(0)          # steps 0-3
            nc.sync.dma_start(out=gb0[:, :], in_=xgb[:, 0:BS])
            # small-overflow rows for steps 1.. -> state slots 0.., one DMA
            nc.sync.dma_start(out=stbig[88:88 + OVS, 0:(_NSTEP - 1) * BS],
                              in_=xgb[:, BS:_NSTEP * BS])
            issue_w(1)
            issue_xc(2)          # kt 4-6   (og3-5)
            nc.sync.dma_start(out=gbig[:, :], in_=xgg[:, :])
            issue_w(2)
            issue_xc(3)          # kt 7-9   (og6-8)
            issue_ga(1)          # steps 4-6
            issue_w(3)
            issue_xc(4)          # kt 10-13 (og9-12)
            issue_ga(2)          # steps 7-9
            issue_w(4)
            issue_xc(5)          # kt 14-17 (og13-17)
            issue_w(5)

            for kind, idx in _EMIT:
                if kind == "og":
                    out_group(idx)
                else:
                    chain_step(idx)

    if not nc.is_finalized():
        nc.finalize()
    return nc


# -------------------------------------------------------------- driver ----

def kernel(inputs: np.ndarray, weights: np.ndarray) -> np.ndarray:
    import ml_dtypes
    from concourse.bass_utils import run_bass_kernel_spmd

    inputs = np.ascontiguousarray(inputs, dtype=np.float32)
    weights = np.ascontiguousarray(weights, dtype=np.float32)

    ops = _build_operators(weights)
    wt_packed = np.ascontiguousarray(_pack_ops(ops)).astype(ml_dtypes.bfloat16)

    # x-major per-sample flatten, then transpose so grid index leads
    xP = inputs.reshape(B, SIZE, SIZE).transpose(0, 2, 1).reshape(B, N)

    nc = _build_device_kernel()
    in_maps = []
    for c in range(NCORES):
        xc = np.ascontiguousarray(xP[c * BS:(c + 1) * BS].T)  # (N, BS) fp32
        xga = np.zeros((P, _NSTEP * BS), dtype=np.float32)
        xgb = np.zeros((OVS, _NSTEP * BS), dtype=np.float32)
        xgg = np.zeros((P - OVS, max(_NBIG, 1) * BS), dtype=np.float32)
        for s in _STEPS:
            i, c0, ng, ka, ov = s["i"], s["c0"], s["ng"], s["ka"], s["ov"]
            r0 = SIZE * c0
            xga[:ka, i * BS:(i + 1) * BS] = xc[r0:r0 + ka]
            if ov:
                if s["use_gb"] and i > 0:
                    xgg[:ov, s["bslot"] * BS:(s["bslot"] + 1) * BS] = \
                        xc[r0 + P:r0 + ng]
                else:
                    xgb[:ov, i * BS:(i + 1) * BS] = xc[r0 + P:r0 + ng]
        # k-tiled layout: [128, NT*BS], k-tile k at free block k
        xTc = np.ascontiguousarray(
            xc.reshape(NT, P, BS).transpose(1, 0, 2).reshape(P, NT * BS)
        )
        in_maps.append({
            "xTc": xTc.astype(ml_dtypes.bfloat16),
            "wt": wt_packed,
            "xga": xga.astype(ml_dtypes.bfloat16),
            "xgb": xgb.astype(ml_dtypes.bfloat16),
            "xgg": xgg.astype(ml_dtypes.bfloat16),
        })
    trace = bool(int(os.environ.get("KERNEL_TRACE", "0")))
    res = run_bass_kernel_spmd(
        nc, in_maps, core_ids=list(range(NCORES)), trace=trace
    )
    if trace and res.exec_time_ns is not None:
        print(f"HW exec time: {res.exec_time_ns} ns")
        if res.instructions_and_trace is not None:
            print(f"trace: {res.instructions_and_trace[1]}")

    outP = np.concatenate(
        [res.results[c]["outT"].astype(np.float32).T for c in range(NCORES)],
        axis=0,
    )
    return np.ascontiguousarray(
        outP.reshape(B, SIZE, SIZE).transpose(0, 2, 1).reshape(B, N)
    )


# revision 22
# speedup vs baseline: 1.0116x; 1.0116x over previous
"""Trainium2 kernel for nn_Graph_41609643163904.

The reference op is a sequential per-cell scatter sweep over a 48x48 grid
(x outer, y inner): read center v, zero it, add v*W[y,x] to the 5x5
neighborhood.  Every step is linear in the grid, so the sweep is a fixed
linear operator M (2304x2304) of the weights.

This version exploits the sweep's sequentially-semiseparable structure:
all influence crossing grid-column X flows through the 88-dim interface
s(X) = [v(X); v(X-1)] (v = per-cell fired values, 44 per column).  In
x-major layout, each 128-row output tile j decomposes EXACTLY as

    out_j = sum_k  nearblk[j,k] @ g0_ktile[k]   (cols > cut_j)
          + U_j @ s(cut_j)                      (everything to the left)

The cut set is globally optimized: instead of one cut per output tile
(16 cuts), a brute-forced 10-cut set shares states between neighboring
tiles, trading a few extra near k-tiles for 6 fewer chain steps.  That
cuts the matmul count 206 -> ~176 (x2 512-wide halves) and the chain g0
DMA stack from 4.2MB to 2.6MB.  All operands bf16 (tolerance 2e-2;
end-to-end lands ~5e-3), halving DMA.  Data-parallel over the 8192-
sample batch across 8 cores, no comm.

Hardware lessons baked into the schedule (each worth ~5-20us):
  * every dma_start costs ~0.7us of its sequencer: loads are merged into
    a few big DMAs; x is pre-tiled on host to [128, 18*BS] so each
    multi-k-tile chunk is ONE dma with contiguous per-partition runs
  * SDMA engines round-robin the *queues* at packet granularity, so
    bytes must be balanced per queue: all loads ride SP (x chunks
    interleaved with chain stacks in first-use order), operator chunks
    ride ACT, stores ride SWDGE -- moving the chain stacks to SWDGE
    starves the x stream (measured +21us)
  * all states live in ONE [104, 10*BS] tile so the per-step overflow
    loads merge into a single DMA (rows 88:104, slot i-1 feeds step i)
  * PSUM->SBUF copies cost ~1.2us nearly flat in size: one big copy per
    PSUM tile, out-copies on DVE, state copies on ACT (disjoint queues)
  * chain step i is emitted right before its first consumer out_group,
    so chain-data DMA latency has maximal slack
  * ~40 dummy matmuls at t=0 keep the PE busy through the HAM 3.4us
    activity window (memset on DVE, not ACT, so warm-up does not wait
    for the activation-table load); first real matmuls then run at
    2.4GHz instead of 1.2
  * final stores ride the by-then-idle SP/ACT HWDGE rings instead of
    queueing behind the SWDGE emission path
"""

import os

import numpy as np

SIZE = 48
D = 2
KS = 5
N = SIZE * SIZE          # 2304
B = 8192
NCORES = 8
BS = B // NCORES         # 1024 samples per core
P = 128
NT = N // P              # 18 tiles of 128
MW = 512                 # matmul moving-dim (PSUM bank)
NM = BS // MW            # 2 halves

# brute-forced cut set (see docstring): 10 cuts shared across out tiles
_XS = [4, 7, 12, 15, 20, 23, 28, 31, 36, 39]
_NSTEP = len(_XS)
OVS = 16                 # small-step overflow rows (ride in the sb piece)


# ---------------------------------------------------------------- plan ----

def _plan():
    js = []
    for j in range(NT):
        r0 = P * j
        jc_lo, jc_hi = r0 // SIZE, (r0 + P - 1) // SIZE
        if j < 2:
            cut, step = None, None
            ncol_lo = 0
        else:
            cut = max(x for x in _XS if x <= jc_lo - 1)
            step = _XS.index(cut)
            ncol_lo = cut + 1
        ncol_hi = min(jc_hi + 2, SIZE - 1)
        kt_lo = (SIZE * ncol_lo) // P
        kt_hi = (SIZE * (ncol_hi + 1) - 1) // P
        js.append(dict(j=j, jc_lo=jc_lo, jc_hi=jc_hi, ncol_lo=ncol_lo,
                       kts=list(range(kt_lo, kt_hi + 1)),
                       cut=cut, step=step))
    steps = []
    nbig = 0
    for i, X in enumerate(_XS):
        c0 = 2 if i == 0 else _XS[i - 1] + 1
        ng = SIZE * (X - c0 + 1)
        ka = min(ng, P)
        ov = max(ng - P, 0)
        big = ov > (0 if i == 0 else 40 - 0)  # i==0 has no sb; use gb there
        # i == 0: overflow always goes to its own gb piece (no sb piece)
        use_gb = (i == 0 and ov > 0) or ov > 40
        bslot = None
        if use_gb and i > 0:
            bslot = nbig
            nbig += 1
        steps.append(dict(i=i, X=X, c0=c0, ng=ng, ka=ka, ov=ov,
                          use_gb=use_gb, bslot=bslot))
    return js, steps, nbig


_JS, _STEPS, _NBIG = _plan()


def _step_pieces(i):
    """chain step i matmul pieces in accumulation order (state last).
    kinds: ga (g0 rows 0:ka), gb (overflow via gb0/gbig), sb (state,
    plus small overflow rows riding in stbig[88:88+ov])."""
    s = _STEPS[i]
    out = [("ga", s["ka"])]
    if s["use_gb"]:
        out.append(("gb", s["ov"]))
    if i > 0:
        k = 88 + (s["ov"] if (s["ov"] and not s["use_gb"]) else 0)
        out.append(("sb", k))
    return out


# ------------------------------------------------------- host operators ----

def _build_M_V(weights):
    """Composed operator M (N,N) and firing-value gradient rows V (1936,N),
    fp64, in the original y-major flattening."""
    M = np.eye(N, dtype=np.float64)
    V = np.zeros((44 * 44, N), dtype=np.float64)
    w = weights.astype(np.float64)
    for x in range(D, SIZE - D):
        for y in range(D, SIZE - D):
            c = y * SIZE + x
            wc = w[y, x]
            rc = M[c].copy()
            V[(x - D) * 44 + (y - D)] = rc
            for dy in range(-D, D + 1):
                r0 = c + dy * SIZE - D
                wrow = wc[dy + D]
                if dy == 0:
                    M[r0:r0 + D] += np.outer(wrow[:D], rc)
                    M[r0 + D + 1:r0 + KS] += np.outer(wrow[D + 1:], rc)
                else:
                    M[r0:r0 + KS] += np.outer(wrow, rc)
            M[c] = wc[D, D] * rc
    return M, V


def _xmajor_idx():
    n = np.arange(N)
    return (n % SIZE) * SIZE + n // SIZE


def _srows(X):
    return np.concatenate([(X - 2) * 44 + np.arange(44),
                           (X - 3) * 44 + np.arange(44)])


def _build_operators(weights):
    M, V = _build_M_V(weights)
    idx = _xmajor_idx()
    Mx = M[np.ix_(idx, idx)]
    Vx = V[:, idx]
    ops = {}
    for d in _JS:
        j = d["j"]
        jr = slice(P * j, P * j + P)
        e_lo = SIZE * d["ncol_lo"]
        for kt in d["kts"]:
            blk = Mx[jr, P * kt:P * kt + P].copy()
            cols = np.arange(P * kt, P * kt + P)
            blk[:, cols < e_lo] = 0.0
            ops[("near", j, kt)] = blk.T        # lhsT (K=128, M=128)
        if d["cut"] is not None:
            X = d["cut"]
            sf = Vx[_srows(X), :SIZE * (X + 1)]
            F = Mx[jr, :SIZE * (X + 1)]
            U, _, _, _ = np.linalg.lstsq(sf.T, F.T, rcond=None)
            ops[("far", j)] = U                 # lhsT (K=88, M=128)
    for s in _STEPS:
        i, X, c0 = s["i"], s["X"], s["c0"]
        Binj = Vx[_srows(X), SIZE * c0:SIZE * (X + 1)]  # full columns
        ka, ov = s["ka"], s["ov"]
        ops[("chain_ga", i)] = Binj.T[:ka]
        if s["use_gb"]:
            ops[("chain_gb", i)] = Binj.T[ka:ka + ov]
        if i > 0:
            Xp = _XS[i - 1]
            sf_p = Vx[_srows(Xp), :SIZE * (Xp + 1)]
            tgt = Vx[_srows(X), :SIZE * (Xp + 1)]
            T, _, _, _ = np.linalg.lstsq(sf_p.T, tgt.T, rcond=None)
            if ov and not s["use_gb"]:
                ops[("chain_sb", i)] = np.vstack([T, Binj.T[ka:ka + ov]])
            else:
                ops[("chain_sb", i)] = T
    return ops


# ------------------------------------------------ emission/consumption ----

def _emission():
    """(kind, idx) sequence: out groups with chain steps spliced right
    before their first consumer."""
    seq = [("og", 0), ("og", 1)]
    done = set()
    for j in range(2, NT):
        st = _JS[j]["step"]
        if st is not None and st not in done:
            # emit any not-yet-emitted steps up to st (chain is sequential)
            for i in range(len(done), st + 1):
                seq.append(("ch", i))
                done.add(i)
        seq.append(("og", j))
    return seq


_EMIT = _emission()


def _pack_layout():
    """Column ranges in the packed wt tensor, in PE consumption order."""
    off = 0
    lay = {}

    def put(key, cols):
        nonlocal off
        lay[key] = (off, cols)
        off += cols

    for kind, idx in _EMIT:
        if kind == "og":
            d = _JS[idx]
            for kt in d["kts"]:
                put(("near", idx, kt), P)
            if d["cut"] is not None:
                put(("far", idx), P)
        else:
            for pc, _k in _step_pieces(idx):
                put((f"chain_{pc}", idx), 88)
    return lay, off


_LAY, _TOTC = _pack_layout()


def _pack_ops(ops):
    wt = np.zeros((P, _TOTC), dtype=np.float32)
    for d in _JS:
        j = d["j"]
        for kt in d["kts"]:
            o, c = _LAY[("near", j, kt)]
            wt[:, o:o + c] = ops[("near", j, kt)]
        if d["cut"] is not None:
            o, c = _LAY[("far", j)]
            wt[:88, o:o + P] = ops[("far", j)]
    for s in _STEPS:
        i = s["i"]
        for pc, k in _step_pieces(i):
            o, _ = _LAY[(f"chain_{pc}", i)]
            blk = ops[(f"chain_{pc}", i)]
            assert blk.shape[0] == k, (i, pc, blk.shape, k)
            wt[:k, o:o + 88] = blk
    return wt


# fetch groups in consumption order: one group per _EMIT entry
def _fetch_groups():
    gs = []
    for kind, idx in _EMIT:
        if kind == "og":
            d = _JS[idx]
            keys = [("near", idx, kt) for kt in d["kts"]]
            if d["cut"] is not None:
                keys.append(("far", idx))
            gs.append((f"og{idx}", keys))
        else:
            gs.append((f"ch{idx}",
                       [(f"chain_{pc}", idx)
                        for pc, _k in _step_pieces(idx)]))
    return gs


_FETCH = _fetch_groups()


# ------------------------------------------------------------- device ----

def _build_device_kernel():
    import concourse.mybir as mybir
    from concourse import bacc
    from concourse.tile import TileContext

    f32 = mybir.dt.float32
    bf16 = mybir.dt.bfloat16

    nc = bacc.Bacc()
    # xTc: host pre-tiled [128, 18*BS] -- k-tile k at free block k, so a
    # multi-k-tile chunk load is one plain 2D slice with contiguous
    # per-partition runs.
    xTc = nc.dram_tensor("xTc", [P, NT * BS], bf16, kind="ExternalInput")
    wt = nc.dram_tensor("wt", [P, _TOTC], bf16, kind="ExternalInput")
    # host-packed chain g0 stacks: step i at free block i
    xga = nc.dram_tensor("xga", [P, _NSTEP * BS], bf16, kind="ExternalInput")
    # small overflow rows (step i block i; zeros for big steps)
    xgb = nc.dram_tensor("xgb", [OVS, _NSTEP * BS], bf16,
                         kind="ExternalInput")
    # big-step overflow rows (112 each), bslot order
    xgg = nc.dram_tensor("xgg", [P - OVS, max(_NBIG, 1) * BS], bf16,
                         kind="ExternalInput")
    outT = nc.dram_tensor("outT", [N, BS], bf16, kind="ExternalOutput")

    XCH = [2, 2, 3, 3, 4, 4]     # x k-tiles per SBUF chunk (one DMA each)
    GACH = [4, 3, 3]             # chain-ga steps per SBUF chunk
    WSIZES = [3, 5, 5, 5, 5, 5]  # w chunk sizes in fetch groups
    NWARM = 40

    with TileContext(nc) as tc:
        with (
            tc.tile_pool(name="xpool", bufs=1) as xpool,
            tc.tile_pool(name="spool", bufs=1) as spool,
            tc.tile_pool(name="wpool", bufs=1) as wpool,
            tc.tile_pool(name="opool", bufs=6) as opool,
            tc.tile_pool(name="pso", bufs=4, space="PSUM") as pso,
            tc.tile_pool(name="pss", bufs=2, space="PSUM") as pss,
        ):
            # Engine / ring roles (SDMA round-robins the rings per packet):
            #   sync  (SP HWDGE):  x chunks + chain stacks, last stores
            #   scalar (ACT HWDGE): operator chunks; state copies
            #   gpsimd (SWDGE):    output stores
            #   vector (DVE):      out PSUM->SBUF copies
            kt0_of = []
            acc = 0
            for nk in XCH:
                kt0_of.append(acc)
                acc += nk
            assert acc == NT

            def chunk_of(kt):
                for ci in range(len(XCH) - 1, -1, -1):
                    if kt >= kt0_of[ci]:
                        return ci
                raise AssertionError

            xch = [xpool.tile([P, nk * BS], bf16, tag=f"x{ci}", name=f"x{ci}")
                   for ci, nk in enumerate(XCH)]

            def issue_xc(ci):
                k0, nk = kt0_of[ci], XCH[ci]
                nc.sync.dma_start(out=xch[ci][:, :],
                                  in_=xTc[:, k0 * BS:(k0 + nk) * BS])

            def x_ap(kt, c0, c1):
                ci = chunk_of(kt)
                off = (kt - kt0_of[ci]) * BS
                return xch[ci][:, off + c0:off + c1]

            ga0_of = []
            acc = 0
            for ns in GACH:
                ga0_of.append(acc)
                acc += ns
            assert acc == _NSTEP

            def gchunk_of(i):
                for ci in range(len(GACH) - 1, -1, -1):
                    if i >= ga0_of[ci]:
                        return ci
                raise AssertionError

            gach = [xpool.tile([P, ns * BS], bf16, tag=f"ga{q}", name=f"ga{q}")
                    for q, ns in enumerate(GACH)]

            def issue_ga(ci):
                s0, ns = ga0_of[ci], GACH[ci]
                nc.sync.dma_start(out=gach[ci][:, :],
                                  in_=xga[:, s0 * BS:(s0 + ns) * BS])

            # step-0 overflow rows (K<=16)
            gb0 = xpool.tile([OVS, BS], bf16, tag="gb0", name="gb0")
            # big-step overflow rows
            gbig = xpool.tile([P - OVS, max(_NBIG, 1) * BS], bf16,
                              tag="gbig", name="gbig")
            # all states in one tile: rows 0:88 = state i at slot i, rows
            # 88:88+OVS = step i+1's small-overflow rows (one merged DMA)
            stbig = spool.tile([88 + OVS, _NSTEP * BS], bf16,
                               tag="st", name="st")

            # operator stream: merged chunk DMAs, consumption order
            wslot = {}
            wchunks = []
            nchunks = len(WSIZES)
            woff = [0]
            for s in WSIZES:
                woff.append(woff[-1] + s)
            assert woff[-1] == len(_FETCH), (woff[-1], len(_FETCH))
            for ci in range(nchunks):
                grp = _FETCH[woff[ci]:woff[ci + 1]]
                keys = [k for _, ks in grp for k in ks]
                o0 = _LAY[keys[0]][0]
                cols = sum(_LAY[k][1] for k in keys)
                wtile = wpool.tile([P, 3584], bf16, tag=f"w{ci}",
                                   name=f"w{ci}")
                wchunks.append((wtile, o0, cols))
                for k in keys:
                    wslot[k] = (wtile, _LAY[k][0] - o0)

            def issue_w(ci, eng=None):
                wtile, o0, cols = wchunks[ci]
                eng = eng or nc.scalar
                eng.dma_start(out=wtile[:, :cols],
                              in_=wt[:, o0:o0 + cols])

            def w_ap(key, kk):
                wtile, o = wslot[key]
                m = 88 if key[0].startswith("chain") else P
                return wtile[0:kk, o:o + m]

            def out_group(j):
                d = _JS[j]
                items = [("near", kt) for kt in d["kts"]]
                if d["cut"] is not None:
                    items.append(("far", None))
                # one 1-bank PSUM tile per 512-half: half-m copies start
                # as soon as that half's accumulation finishes
                psm = [pso.tile([P, MW], f32, tag="o", name=f"ps{j}_{m}")
                       for m in range(NM)]
                s0 = (d["step"] or 0) * BS
                for it, (kind, kt) in enumerate(items):
                    first, last = it == 0, it == len(items) - 1
                    for m in range(NM):
                        if kind == "near":
                            lhsT = w_ap(("near", j, kt), P)
                            rhs = x_ap(kt, m * MW, (m + 1) * MW)
                        else:
                            lhsT = w_ap(("far", j), 88)
                            rhs = stbig[0:88, s0 + m * MW:s0 + (m + 1) * MW]
                        nc.tensor.matmul(psm[m][:, :],
                                         lhsT=lhsT, rhs=rhs,
                                         start=first, stop=last)
                oc = opool.tile([P, BS], bf16, tag="o", name=f"oc{j}")
                if j == NT - 1:  # final tile: parallel half copies + stores
                    nc.vector.tensor_copy(oc[:, 0:MW], psm[0][:, :])
                    nc.scalar.copy(oc[:, MW:BS], psm[1][:, :])
                    nc.sync.dma_start(out=outT[P * j:P * j + P, 0:MW],
                                      in_=oc[:, 0:MW])
                    nc.scalar.dma_start(out=outT[P * j:P * j + P, MW:BS],
                                        in_=oc[:, MW:BS])
                else:
                    nc.vector.tensor_copy(oc[:, 0:MW], psm[0][:, :])
                    nc.vector.tensor_copy(oc[:, MW:BS], psm[1][:, :])
                    if j >= NT - 3:
                        nc.sync.dma_start(out=outT[P * j:P * j + P, 0:MW],
                                          in_=oc[:, 0:MW])
                        nc.sync.dma_start(out=outT[P * j:P * j + P, MW:BS],
                                          in_=oc[:, MW:BS])
                    else:
                        nc.gpsimd.dma_start(out=outT[P * j:P * j + P, :],
                                            in_=oc[:])

            def chain_step(i):
                s = _STEPS[i]
                ps = pss.tile([88, BS], f32, tag="s", name=f"pss{i}")
                pieces = _step_pieces(i)
                for it, (pc, kdim) in enumerate(pieces):
                    first, last = it == 0, it == len(pieces) - 1
                    if pc == "ga":
                        ci = gchunk_of(i)
                        rt, base = gach[ci], (i - ga0_of[ci]) * BS
                    elif pc == "gb":
                        if i == 0:
                            rt, base = gb0, 0
                        else:
                            rt, base = gbig, s["bslot"] * BS
                    else:
                        rt, base = stbig, (i - 1) * BS
                    for m in range(NM):
                        nc.tensor.matmul(
                            ps[:, m * MW:(m + 1) * MW],
                            lhsT=w_ap((f"chain_{pc}", i), kdim),
                            rhs=rt[0:kdim, base + m * MW:base + (m + 1) * MW],
                            start=first, stop=last)
                nc.scalar.copy(stbig[0:88, i * BS:(i + 1) * BS], ps[:])

            # ---------------- emission ----------------
            # PE warm-up: un-throttle HAM during the initial DMA window.
            # memset on DVE so warm-up does not wait for the ACT
            # activation-table load.
            wu = spool.tile([P, P], bf16, tag="warm", name="warm")
            nc.vector.memset(wu[:], 0.0)
            pwu = pso.tile([P, MW], f32, tag="o", name="pswarm")
            for _ in range(NWARM):
                nc.tensor.matmul(pwu[:, 0:64], lhsT=wu[:], rhs=wu[:, 0:64],
                                 start=True, stop=True)

            # Loads.  The SP and ACT HWDGE rings generate descriptors
            # concurrently (~0.65us per dma_start each), but only 8 HWDGE
            # sem lanes exist globally: the (8+k)th HWDGE dma_start gates
            # on the k-th one's first consumer.  So the x / chain-ga
            # stream rides SP in consumption order, the operator chunks
            # ride ACT, and the chain-overflow stacks ride the SWDGE ring
            # (own generator + own sems, idle until the first store).
            issue_xc(0)          # kt 0-1   (og0)
            issue_w(0)
            issue_xc(1)          # kt 2-3   (og1-2)
            issue_ga(0)          # steps 0-3
            nc.sync.dma_start(out=gb0[:, :], in_=xgb[:, 0:BS])
            # small-overflow rows for steps 1.. -> state slots 0.., one DMA
            nc.sync.dma_start(out=stbig[88:88 + OVS, 0:(_NSTEP - 1) * BS],
                              in_=xgb[:, BS:_NSTEP * BS])
            issue_w(1)
            issue_xc(2)          # kt 4-6   (og3-5)
            nc.sync.dma_start(out=gbig[:, :], in_=xgg[:, :])
            issue_w(2)
            issue_xc(3)          # kt 7-9   (og6-8)
            issue_ga(1)          # steps 4-6
            issue_w(3)
            issue_xc(4)          # kt 10-13 (og9-12)
            issue_ga(2)          # steps 7-9
            issue_w(4)
            issue_xc(5)          # kt 14-17 (og13-17)
            issue_w(5)

            for kind, idx in _EMIT:
                if kind == "og":
                    out_group(idx)
                else:
                    chain_step(idx)

    if not nc.is_finalized():
        nc.finalize()
    return nc


# -------------------------------------------------------------- driver ----

def kernel(inputs: np.ndarray, weights: np.ndarray) -> np.ndarray:
    import ml_dtypes
    from concourse.bass_utils import run_bass_kernel_spmd

    inputs = np.ascontiguousarray(inputs, dtype=np.float32)
    weights = np.ascontiguousarray(weights, dtype=np.float32)

    ops = _build_operators(weights)
    wt_packed = np.ascontiguousarray(_pack_ops(ops)).astype(ml_dtypes.bfloat16)

    # x-major per-sample flatten, then transpose so grid index leads
    xP = inputs.reshape(B, SIZE, SIZE).transpose(0, 2, 1).reshape(B, N)

    nc = _build_device_kernel()
    in_maps = []
    for c in range(NCORES):
        xc = np.ascontiguousarray(xP[c * BS:(c + 1) * BS].T)  # (N, BS) fp32
        xga = np.zeros((P, _NSTEP * BS), dtype=np.float32)
        xgb = np.zeros((OVS, _NSTEP * BS), dtype=np.float32)
        xgg = np.zeros((P - OVS, max(_NBIG, 1) * BS), dtype=np.float32)
        for s in _STEPS:
            i, c0, ng, ka, ov = s["i"], s["c0"], s["ng"], s["ka"], s["ov"]
            r0 = SIZE * c0
            xga[:ka, i * BS:(i + 1) * BS] = xc[r0:r0 + ka]
            if ov:
                if s["use_gb"] and i > 0:
                    xgg[:ov, s["bslot"] * BS:(s["bslot"] + 1) * BS] = \
                        xc[r0 + P:r0 + ng]
                else:
                    xgb[:ov, i * BS:(i + 1) * BS] = xc[r0 + P:r0 + ng]
        # k-tiled layout: [128, NT*BS], k-tile k at free block k
        xTc = np.ascontiguousarray(
            xc.reshape(NT, P, BS).transpose(1, 0, 2).reshape(P, NT * BS)
        )
        in_maps.append({
            "xTc": xTc.astype(ml_dtypes.bfloat16),
            "wt": wt_packed,
            "xga": xga.astype(ml_dtypes.bfloat16),
            "xgb": xgb.astype(ml_dtypes.bfloat16),
            "xgg": xgg.astype(ml_dtypes.bfloat16),
        })
    trace = bool(int(os.environ.get("KERNEL_TRACE", "0")))
    res = run_bass_kernel_spmd(
        nc, in_maps, core_ids=list(range(NCORES)), trace=trace
    )
    if trace and res.exec_time_ns is not None:
        print(f"HW exec time: {res.exec_time_ns} ns")
        if res.instructions_and_trace is not None:
            print(f"trace: {res.instructions_and_trace[1]}")

    outP = np.concatenate(
        [res.results[c]["outT"].astype(np.float32).T for c in range(NCORES)],
        axis=0,
    )
    return np.ascontiguousarray(
        outP.reshape(B, SIZE, SIZE).transpose(0, 2, 1).reshape(B, N)
    )
